# revision 1
# baseline (speedup 1.0000x reference)
"""Trainium2 Bass kernel for the Capsule routing layer (nn_Capsule_49658411876931).

Math (see reference):
    u_hat[b,j,i,d] = sum_k W[j,i,d,k] * x[b,i,k]
    b0 = 0
    for r in 0..2:
        c = softmax(b, axis=j)
        s[b,j,d] = sum_i c[b,j,i] u_hat[b,j,i,d]
        v = squash(s)  (over d)
        if r < 2: b += sum_d u_hat[b,j,i,d] v[b,j,d]
    return v  [B, J, D]

Sharding: input-capsule axis I=2048 split over 8 cores (I_LOC=256). W is
I-sharded (4.2 MB/core instead of 33 MB replicated). Softmax over J is
core-local; the only cross-core communication is an AllReduce of the
partial s [B, J*D] = 64 KB per routing iteration.

Per-core layouts (P = SBUF partition index):
  i_local = g*16 + r*4 + c   (g in 0..15, r,c in 0..3)
  u_hat "C" tensor : [P = 32*c + b, free = (g, r, d, j)]  bf16
  b-logits / c     : [P = 32*c + b, free = (g, r, j)]
u_hat is computed with 16-way tile_position-packed PE matmuls
(stationary x_i [k=8, b=32] at array tile (r,c), moving W_i [k=8, jd=512]).
Cross-partition sums (collapse of the 4 c-strips, v broadcast) use small
PE matmuls with 0/1 selector matrices (DVE lanes cannot cross partitions).
"""

import numpy as np
import ml_dtypes

import concourse.bass as bass
import concourse.tile as tile
from concourse import bacc, mybir
from concourse.bass_utils import run_bass_kernel_spmd

F32 = mybir.dt.float32
BF16 = mybir.dt.float16  # fp16: 11-bit mantissa, ample range here
U32 = mybir.dt.uint32
Alu = mybir.AluOpType
Act = mybir.ActivationFunctionType

B, I, K = 32, 2048, 8
J, D = 32, 16
JD = J * D                     # 512
NCORES = 8
I_LOC = I // NCORES            # 256
NG = I_LOC // 16               # 16 groups of 16 input capsules per core
ROUTINGS = 3
EPS = 1e-7

_CACHE = {}
import os
DEBUG_STAGE = os.environ.get("KSTAGE", "")


def _build():
    nc = bacc.Bacc("TRN2", target_bir_lowering=False, debug=False, num_devices=NCORES)

    wt_in = nc.dram_tensor("wt", [NG, 4, 8, 4, JD], F32, kind="ExternalInput")
    xs_in = nc.dram_tensor("xs", [4, 8, NG, 4, B], F32, kind="ExternalInput")
    v_out = nc.dram_tensor("v", [B, J, D], F32, kind="ExternalOutput")

    # Constant block: selector matrices for cross-partition PE ops plus
    # uint32 constants for the fast-inverse-sqrt, packed into one tensor so
    # a single DMA (one wait) covers all of them.
    # sel[p, b'] = 1 iff p % 32 == b'   (collapse the 4 c-strips)
    sel_np = np.zeros((128, B), np.float32)
    sel_np[np.arange(128), np.arange(128) % B] = 1.0
    consts_np = np.zeros((128, 224), np.float32)
    consts_np[:, 0:32] = sel_np
    consts_np[0:B, 32:160] = sel_np.T          # selT[b, p]
    consts_np[0:B, 160:192] = np.full((B, J), 0x5F3759DF, np.uint32).view(np.float32)
    consts_np[0:B, 192:224] = np.ones((B, J), np.uint32).view(np.float32)
    consts_dram = nc.inline_tensor(consts_np, "consts")

    with tile.TileContext(nc) as tc:
        with (
            tc.tile_pool(name="persist", bufs=1) as pp,
            tc.tile_pool(name="small", bufs=1) as sp,
            tc.tile_pool(name="dram", bufs=1, space="DRAM") as dp,
        ):
            # ---- persistent SBUF tensors ----
            xs = pp.tile([128, NG, 4, B], F32)          # x stationary, rows 32r+k
            C = pp.tile([128, NG, 4, D, J], BF16)       # u_hat
            bl = pp.tile([128, NG, 4, J], F32)          # routing logits
            c_sb = pp.tile([128, NG, 4, J], BF16)       # softmax coefficients
            p_t = pp.tile([128, NG, 4, J], F32)         # exp(b)
            consts = pp.tile([128, 224], F32)
            v_rep = pp.tile([128, D, J], BF16)          # v replicated over c-strips

            sel = consts[:, 0:32]
            selT = consts[0:B, 32:160]
            magic = consts[0:B, 160:192].bitcast(U32)
            oneu = consts[0:B, 192:224].bitcast(U32)

            nc.sync.dma_start(consts[:], consts_dram[:])
            for r in range(4):
                nc.sync.dma_start(xs[32 * r : 32 * r + 8], xs_in[r])
            nc.vector.memset(bl[:], 0.0)
            # Funnel all initial-load waits through one barrier so the first
            # matmuls don't exceed the per-instruction sync-wait budget.
            tc.strict_bb_all_engine_barrier()

            # ---- phase 1: u_hat ----
            with (
                tc.tile_pool(name="wpool", bufs=3) as wp,
                tc.tile_pool(name="psum1", bufs=2, space="PSUM") as ps1,
            ):
                for g in range(NG):
                    wt_g = wp.tile([128, 4, JD], F32, tag="wt")
                    for r in range(4):
                        nc.sync.dma_start(wt_g[32 * r : 32 * r + 8], wt_in[g, r])
                    ps = ps1.tile([128, 4, JD], F32, tag="ps")
                    for r in range(4):
                        for c in range(4):
                            nc.tensor.matmul(
                                ps[32 * c : 32 * c + 32, r, :],
                                xs[32 * r : 32 * r + 8, g, c, :],
                                wt_g[32 * r : 32 * r + 8, c, :],
                                tile_position=(32 * r, 32 * c),
                            )
                    # evacuate [128, (r, jd)] -> C[:, g, (r, d, j)] bf16
                    src = ps.rearrange("p r (j d) -> p r d j", j=J, d=D)
                    if g % 2 == 0:
                        nc.scalar.copy(C[:, g], src)
                    else:
                        nc.vector.tensor_copy(C[:, g], src)

            if DEBUG_STAGE == "phase1":
                dbg = sp.tile([B, J, D], F32, tag="dbg")
                nc.vector.tensor_copy(
                    dbg[:], C[0:B, 0, 0].rearrange("p d j -> p j d")
                )
                nc.sync.dma_start(v_out[:], dbg[:])
            # ---- routing ----
            skip_routing = DEBUG_STAGE == "phase1"
            with (
                tc.tile_pool(name="pipool", bufs=1) as pip,
                tc.tile_pool(name="psum2", bufs=2, space="PSUM") as ps2,
            ):
                for it in range(ROUTINGS if not skip_routing else 0):
                    if it == 0:
                        nc.vector.memset(c_sb[:], 1.0 / J)
                    else:
                        nc.scalar.activation(p_t[:], bl[:], Act.Exp)
                        S = sp.tile([128, NG, 4], F32, tag="S")
                        nc.vector.tensor_reduce(
                            S[:], p_t[:], axis=mybir.AxisListType.X, op=Alu.add
                        )
                        Sr = sp.tile([128, NG, 4], F32, tag="Sr")
                        nc.vector.reciprocal(Sr[:], S[:])
                        nc.vector.tensor_tensor(
                            c_sb[:],
                            p_t[:],
                            Sr[:, :, :, None].broadcast_to([128, NG, 4, J]),
                            op=Alu.mult,
                        )

                    # s partial: pi = C * c (bcast over d); reduce over (g, r)
                    pi = pip.tile([128, NG, 4, D, J], BF16, tag="pi")
                    nc.vector.tensor_tensor(
                        pi[:],
                        C[:],
                        c_sb[:, :, :, None, :].broadcast_to([128, NG, 4, D, J]),
                        op=Alu.mult,
                    )
                    s_red = sp.tile([128, D, J], F32, tag="s_red")
                    nc.vector.tensor_reduce(
                        s_red[:],
                        pi.rearrange("p g r d j -> p (d j) (g r)"),
                        axis=mybir.AxisListType.X,
                        op=Alu.add,
                    )
                    # collapse the 4 c-strips on the PE: s32 = sel^T @ s_red
                    s_ps = ps2.tile([B, D * J], F32, tag="s_ps")
                    nc.tensor.matmul(
                        s_ps[:], sel, s_red.rearrange("p d j -> p (d j)")
                    )
                    s_loc = sp.tile([B, D * J], F32, tag="s_loc")
                    nc.scalar.copy(s_loc[:], s_ps[:])

                    # AllReduce partial s over the 8 cores
                    cc_in = dp.tile([B, D * J], F32, tag="cc_in")
                    cc_out = dp.tile(
                        [B, D * J], F32, tag="cc_out", addr_space="Shared"
                    )
                    s_glob = sp.tile([B, D, J], F32, tag="s_glob")
                    if DEBUG_STAGE == "nocc":
                        nc.vector.tensor_copy(
                            s_glob.rearrange("b d j -> b (d j)"), s_loc[:]
                        )
                    else:
                        nc.gpsimd.dma_start(cc_in[:], s_loc[:])
                        nc.gpsimd.collective_compute(
                            "AllReduce",
                            Alu.add,
                            replica_groups=[list(range(NCORES))],
                            ins=[cc_in.opt()],
                            outs=[cc_out.opt()],
                        )
                        nc.gpsimd.dma_start(
                            s_glob.rearrange("b d j -> b (d j)"), cc_out[:]
                        )

                    # ---- squash on [B, D, J] (all cores redundantly) ----
                    sq = sp.tile([B, D, J], F32, tag="sq")
                    nc.vector.tensor_tensor(sq[:], s_glob[:], s_glob[:], op=Alu.mult)
                    n2 = sp.tile([B, J], F32, tag="n2")
                    nc.vector.tensor_reduce(
                        n2[:],
                        sq.rearrange("b d j -> b j d"),
                        axis=mybir.AxisListType.X,
                        op=Alu.add,
                    )
                    n2e = sp.tile([B, J], F32, tag="n2e")
                    nc.vector.tensor_scalar_add(n2e[:], n2[:], EPS)
                    # fast inverse sqrt + 3 Newton steps (DVE only, no ACT tables)
                    xh = sp.tile([B, J], F32, tag="xh")
                    nc.vector.tensor_scalar_mul(xh[:], n2e[:], 0.5)
                    rsq = sp.tile([B, J], F32, tag="rsq")
                    tmp = sp.tile([B, J], F32, tag="tmp")
                    nc.vector.tensor_tensor(
                        tmp.bitcast(U32), n2e.bitcast(U32), oneu,
                        op=Alu.logical_shift_right,
                    )
                    nc.vector.tensor_tensor(
                        rsq.bitcast(U32), magic, tmp.bitcast(U32), op=Alu.subtract
                    )
                    for _ in range(3):
                        nc.vector.tensor_tensor(tmp[:], rsq[:], rsq[:], op=Alu.mult)
                        nc.vector.tensor_tensor(tmp[:], xh[:], tmp[:], op=Alu.mult)
                        nc.vector.tensor_scalar(
                            tmp[:], tmp[:], -1.0, 1.5, op0=Alu.mult, op1=Alu.add
                        )
                        nc.vector.tensor_tensor(rsq[:], rsq[:], tmp[:], op=Alu.mult)
                    # factor = n2 / (1 + n2) * rsq
                    fac = sp.tile([B, J], F32, tag="fac")
                    nc.vector.tensor_scalar_add(tmp[:], n2[:], 1.0)
                    nc.vector.reciprocal(fac[:], tmp[:])
                    nc.vector.tensor_tensor(fac[:], fac[:], n2[:], op=Alu.mult)
                    nc.vector.tensor_tensor(fac[:], fac[:], rsq[:], op=Alu.mult)
                    v_f = sp.tile([B, D, J], F32, tag="v_f")
                    nc.vector.tensor_tensor(
                        v_f[:],
                        s_glob[:],
                        fac[:, None, :].broadcast_to([B, D, J]),
                        op=Alu.mult,
                    )

                    if it < ROUTINGS - 1:
                        # replicate v over the 4 c-strips via PE, then agreement
                        vr_ps = ps2.tile([128, D * J], F32, tag="vr_ps")
                        nc.tensor.matmul(
                            vr_ps[:], selT, v_f.rearrange("b d j -> b (d j)")
                        )
                        nc.scalar.copy(
                            v_rep.rearrange("p d j -> p (d j)"), vr_ps[:]
                        )
                        pi2 = pip.tile([128, NG, 4, D, J], BF16, tag="pi")
                        nc.vector.tensor_tensor(
                            pi2[:],
                            C[:],
                            v_rep[:, None, None, :, :].broadcast_to(
                                [128, NG, 4, D, J]
                            ),
                            op=Alu.mult,
                        )
                        a_t = sp.tile([128, NG, 4, J], F32, tag="a_t")
                        nc.vector.tensor_reduce(
                            a_t[:],
                            pi2.rearrange("p g r d j -> p g r j d"),
                            axis=mybir.AxisListType.X,
                            op=Alu.add,
                        )
                        nc.vector.tensor_add(bl[:], bl[:], a_t[:])
                    else:
                        # final output: reorder (d, j) -> (j, d) and store
                        v_jd = sp.tile([B, J, D], F32, tag="v_jd")
                        nc.vector.tensor_copy(
                            v_jd[:], v_f.rearrange("b d j -> b j d")
                        )
                        nc.sync.dma_start(v_out[:], v_jd[:])

    nc.compile()
    return nc


def _prep_inputs(x, W):
    """Per-core host-side sharding + layout prep (fp32)."""
    in_maps = []
    for m in range(NCORES):
        lo, hi = m * I_LOC, (m + 1) * I_LOC
        Wc = W[:, lo:hi]                       # [J, 256, D, K]
        Wc = Wc.reshape(J, NG, 4, 4, D, K)     # i = g*16 + r*4 + c
        # -> [g, r, k, c, j, d]
        wt = np.ascontiguousarray(Wc.transpose(1, 2, 5, 3, 0, 4)).reshape(
            NG, 4, 8, 4, JD
        )
        xc = x[:, lo:hi, :].reshape(B, NG, 4, 4, K)
        xs = np.ascontiguousarray(xc.transpose(2, 4, 1, 3, 0))  # [r, k, g, c, b]
        in_maps.append(
            {"wt": wt.astype(np.float32), "xs": xs.astype(np.float32)}
        )
    return in_maps


def run(inputs, trace=False):
    if "nc" not in _CACHE:
        _CACHE["nc"] = _build()
    nc = _CACHE["nc"]
    in_maps = _prep_inputs(inputs["x"], inputs["W"])
    bkr = run_bass_kernel_spmd(
        nc, in_maps, core_ids=list(range(NCORES)), trace=trace
    )
    out = bkr.results[0]["v"].astype(np.float32)
    return out, bkr


def kernel(x, W):
    out, _ = run({"x": np.asarray(x), "W": np.asarray(W)})
    return out



# revision 8
# speedup vs baseline: 2.3110x; 2.3110x over previous
"""Trainium2 Bass kernel for the Capsule routing layer (nn_Capsule_49658411876931).

Math (see reference):
    u_hat[b,j,i,d] = sum_k W[j,i,d,k] * x[b,i,k]
    b0 = 0
    for r in 0..2:
        c = softmax(b, axis=j)
        s[b,j,d] = sum_i c[b,j,i] u_hat[b,j,i,d]
        v = squash(s)  (over d)
        if r < 2: b += sum_d u_hat[b,j,i,d] v[b,j,d]
    return v  [B, J, D]

Sharding: input-capsule axis I=2048 split over 8 cores (I_LOC=256).  The only
cross-core communication is an AllReduce of the partial s [B, J*D] = 64 KB per
routing iteration.

Per-core layouts (P = SBUF partition index), i_local = g*16 + r*4 + c:
  u_hat "C"  : [P = 32*c + b, free = (g, r, d, j)]  fp16
  logits bl  : [P = 32*c + b, free = (g, r, j)]     fp32, lives in PSUM
All big reductions run on the PE array (cheap), not the DVE:
  - iter-0 s: direct matmul over (i,k) chunks: lhsT = x chunks [128, b],
    moving = W chunks [128, (d,j)], PSUM-accumulated (c is uniform 1/J).
  - s-step: lhsT = strip-collapse selector [128, 32], moving = pi slices
    [128, (d,j)] per (g,r), PSUM-accumulated over the 64 slices.
  - agreement: lhsT = identity [128, 128], moving = pi2 slices [128, (g,r,j)]
    per d, PSUM-accumulated over d directly into the logits PSUM region
    (accumulates across routing iterations too: b += ...).
  - phase-1 u_hat: contraction over (c,k)=32 rows at 4 row-strip tile
    positions (r), stationary = c'-selector-expanded x [32, 128], moving =
    W [32, (d,j)]; W passes through the PE exactly once, as fp16.
The remaining DVE work is the per-sample elementwise mults (pi = C*c,
pi2 = C*v) at 2x fp16 throughput, plus softmax/squash small ops.
"""

import numpy as np
import ml_dtypes

import concourse.bass as bass
import concourse.tile as tile
from concourse import bacc, mybir
from concourse.bass_utils import run_bass_kernel_spmd

F32 = mybir.dt.float32
F16 = mybir.dt.float16  # fp16: 11-bit mantissa, ample range here
U32 = mybir.dt.uint32
Alu = mybir.AluOpType
Act = mybir.ActivationFunctionType

B, I, K = 32, 2048, 8
J, D = 32, 16
JD = J * D                     # 512
NCORES = 8
I_LOC = I // NCORES            # 256
NG = I_LOC // 16               # 16 groups of 16 input capsules per core
NCH = I_LOC * K // 128         # 16 contraction chunks of 128 for (i,k)
ROUTINGS = 3
EPS = 1e-7

_CACHE = {}
import os
DEBUG_STAGE = os.environ.get("KSTAGE", "")


def _build():
    nc = bacc.Bacc("TRN2", target_bir_lowering=False, debug=False, num_devices=NCORES)

    wm_in = nc.dram_tensor("wm", [128, NG, JD], F16, kind="ExternalInput")
    ws0_in = nc.dram_tensor("ws0", [128, NCH, JD], F16, kind="ExternalInput")
    xsel_in = nc.dram_tensor("xsel", [128, NG, 128], F16, kind="ExternalInput")
    xs0_in = nc.dram_tensor("xs0", [128, NCH, B], F16, kind="ExternalInput")
    v_out = nc.dram_tensor("v", [B, J, D], F32, kind="ExternalOutput")

    # f32 constants: selT (v broadcast), rsqrt magic numbers
    cf32_np = np.zeros((128, 192), np.float32)
    selT_np = np.zeros((B, 128), np.float32)
    selT_np[np.arange(128) % B, np.arange(128)] = 1.0
    cf32_np[0:B, 0:128] = selT_np
    cf32_np[0:B, 128:160] = np.full((B, J), 0x5F3759DF, np.uint32).view(np.float32)
    cf32_np[0:B, 160:192] = np.full((B, J), 1, np.uint32).view(np.float32)
    cf32_dram = nc.inline_tensor(cf32_np, "cf32")

    # f16 constants: sel (strip collapse) + identity (d-accumulate pass-through)
    cf16_np = np.zeros((128, 160), np.float16)
    cf16_np[np.arange(128), np.arange(128) % B] = 1.0           # sel [128, 32]
    cf16_np[np.arange(128), 32 + np.arange(128)] = 1.0          # ident [128,128]
    cf16_dram = nc.inline_tensor(cf16_np, "cf16")

    with tile.TileContext(nc) as tc:
        with (
            tc.tile_pool(name="persist", bufs=1) as pp,
            tc.tile_pool(name="small", bufs=1) as sp,
            tc.tile_pool(name="dram", bufs=1, space="DRAM") as dp,
            tc.tile_pool(name="psA", bufs=1, space="PSUM") as psA,
        ):
            # ---- persistent SBUF tensors ----
            C = pp.tile([128, NG, 4, D, J], F16)        # u_hat
            p_t = pp.tile([128, NG, 4, J], F32)         # exp(bl)
            c_sb = pp.tile([128, NG, 4, J], F16)        # softmax coefficients
            v_rep = pp.tile([128, D, J], F16)           # v replicated to all partitions
            ws0 = pp.tile([128, NCH, JD], F16)
            xs0 = pp.tile([128, NCH, B], F16)
            xsel = pp.tile([128, NG, 128], F16)
            cf32 = pp.tile([128, 192], F32)
            cf16 = pp.tile([128, 160], F16)

            selT = cf32[0:B, 0:128]
            magic = cf32[0:B, 128:160].bitcast(U32)
            oneu = cf32[0:B, 160:192].bitcast(U32)
            sel = cf16[:, 0:32]
            ident = cf16[:, 32:160]

            nc.sync.dma_start(cf32[:], cf32_dram[:])
            nc.sync.dma_start(cf16[:], cf16_dram[:])
            nc.sync.dma_start(xs0[:], xs0_in[:])
            nc.sync.dma_start(xsel[:], xsel_in[:])
            nc.sync.dma_start(ws0[:], ws0_in[:])
            tc.strict_bb_all_engine_barrier()

            # ---- iter-0 s: direct matmul, c uniform (1/J folded into xs0) ----
            s_ps = psA.tile([B, JD], F32, tag="s_ps")
            for ch in range(NCH):
                nc.tensor.matmul(
                    s_ps[:], xs0[:, ch, :], ws0[:, ch, :],
                    start=(ch == 0), stop=(ch == NCH - 1),
                )
            s_loc = sp.tile([B, JD], F32, tag="s_loc")
            nc.scalar.copy(s_loc[:], s_ps[:])

            def all_reduce_s(it):
                cc_in = dp.tile([B, JD], F32, tag="cc_in")
                cc_out = dp.tile([B, JD], F32, tag="cc_out", addr_space="Shared")
                s_glob = sp.tile([B, D, J], F32, tag="s_glob")
                nc.gpsimd.dma_start(cc_in[:], s_loc[:])
                nc.gpsimd.collective_compute(
                    "AllReduce",
                    Alu.add,
                    replica_groups=[list(range(NCORES))],
                    ins=[cc_in.opt()],
                    outs=[cc_out.opt()],
                )
                nc.gpsimd.dma_start(
                    s_glob.rearrange("b d j -> b (d j)"), cc_out[:]
                )
                return s_glob

            s_glob = all_reduce_s(0)

            # ---- phase 1 (overlaps the AllReduce): u_hat via (c,k)-contraction
            # 64 matmuls: stationary xsel[32r:32r+32, g, :] (zero except c==c'
            # block), moving wm[32r:32r+32, g, :]; 4 r-strips run concurrently
            # on distinct row-groups of the PE array.
            with (
                tc.tile_pool(name="wq", bufs=2) as wq,
                tc.tile_pool(name="ph1", bufs=2, space="PSUM") as ph1,
            ):
                for gq in range(4):
                    wm_q = wq.tile([128, 4, JD], F16, tag="wm")
                    nc.sync.dma_start(wm_q[:], wm_in[:, 4 * gq : 4 * gq + 4, :])
                    for gi in range(4):
                        g = 4 * gq + gi
                        for rp in range(2):
                            ps1 = ph1.tile([128, 2, JD], F32, tag="ps1")
                            for rr in range(2):
                                r = 2 * rp + rr
                                nc.tensor.matmul(
                                    ps1[:, rr, :],
                                    xsel[32 * r : 32 * r + 32, g, :],
                                    wm_q[32 * r : 32 * r + 32, gi, :],
                                    tile_position=(32 * r, 0),
                                )
                            # evacuate [128, 2, (d,j)] -> C[:, g, 2rp:2rp+2, :, :]
                            dst = C[:, g, 2 * rp : 2 * rp + 2].rearrange(
                                "p r d j -> p r (d j)"
                            )
                            if (2 * g + rp) % 5 < 3:
                                nc.scalar.copy(dst, ps1[:])
                            else:
                                nc.vector.tensor_copy(dst, ps1[:])

            if DEBUG_STAGE == "phase1":
                dbg = sp.tile([B, J, D], F32, tag="dbg")
                nc.vector.tensor_copy(
                    dbg[:], C[0:B, 0, 0].rearrange("p d j -> p j d")
                )
                nc.sync.dma_start(v_out[:], dbg[:])

            # ---- squash helper: s_glob [B, D, J] f32 -> v_f [B, D, J] f32 ----
            def squash(s_glob):
                sq = sp.tile([B, D, J], F32, tag="sq")
                nc.vector.tensor_tensor(sq[:], s_glob[:], s_glob[:], op=Alu.mult)
                n2 = sp.tile([B, J], F32, tag="n2")
                nc.vector.tensor_reduce(
                    n2[:],
                    sq.rearrange("b d j -> b j d"),
                    axis=mybir.AxisListType.X,
                    op=Alu.add,
                )
                n2e = sp.tile([B, J], F32, tag="n2e")
                nc.vector.tensor_scalar_add(n2e[:], n2[:], EPS)
                # fast inverse sqrt + 3 Newton steps (DVE only, no ACT tables)
                xh = sp.tile([B, J], F32, tag="xh")
                nc.vector.tensor_scalar_mul(xh[:], n2e[:], 0.5)
                rsq = sp.tile([B, J], F32, tag="rsq")
                tmp = sp.tile([B, J], F32, tag="tmp")
                nc.vector.tensor_tensor(
                    tmp.bitcast(U32), n2e.bitcast(U32), oneu,
                    op=Alu.logical_shift_right,
                )
                nc.vector.tensor_tensor(
                    rsq.bitcast(U32), magic, tmp.bitcast(U32), op=Alu.subtract
                )
                for _ in range(3):
                    nc.vector.tensor_tensor(tmp[:], rsq[:], rsq[:], op=Alu.mult)
                    nc.vector.tensor_tensor(tmp[:], xh[:], tmp[:], op=Alu.mult)
                    nc.vector.tensor_scalar(
                        tmp[:], tmp[:], -1.0, 1.5, op0=Alu.mult, op1=Alu.add
                    )
                    nc.vector.tensor_tensor(rsq[:], rsq[:], tmp[:], op=Alu.mult)
                # factor = n2 / (1 + n2) * rsq
                fac = sp.tile([B, J], F32, tag="fac")
                nc.vector.tensor_scalar_add(tmp[:], n2[:], 1.0)
                nc.vector.reciprocal(fac[:], tmp[:])
                nc.vector.tensor_tensor(fac[:], fac[:], n2[:], op=Alu.mult)
                nc.vector.tensor_tensor(fac[:], fac[:], rsq[:], op=Alu.mult)
                v_f = sp.tile([B, D, J], F32, tag="v_f")
                nc.vector.tensor_tensor(
                    v_f[:],
                    s_glob[:],
                    fac[:, None, :].broadcast_to([B, D, J]),
                    op=Alu.mult,
                )
                return v_f

            with (
                tc.tile_pool(name="prod", bufs=3) as prod,
                tc.tile_pool(name="psB", bufs=2, space="PSUM") as psB,
                tc.tile_pool(name="psC", bufs=1, space="PSUM") as psC,
            ):
                # persistent routing logits bl [128, (g, r, j)] = 4 PSUM banks
                bl_ps = psC.tile([128, NG, 4, J], F32)
                for it in range(ROUTINGS):
                    # ---- softmax + s-step (iters 1, 2; iter 0 done above) ----
                    if it > 0:
                        nc.scalar.activation(p_t[:], bl_ps[:], Act.Exp)
                        S = sp.tile([128, NG, 4], F32, tag="S")
                        nc.vector.tensor_reduce(
                            S[:], p_t[:], axis=mybir.AxisListType.X, op=Alu.add
                        )
                        Sr = sp.tile([128, NG, 4], F32, tag="Sr")
                        nc.vector.reciprocal(Sr[:], S[:])
                        nc.vector.tensor_tensor(
                            c_sb[:],
                            p_t[:],
                            Sr[:, :, :, None].broadcast_to([128, NG, 4, J]),
                            op=Alu.mult,
                        )
                        # pi = C * c (bcast over d), per-g; s-step matmuls
                        # accumulate over the 64 (g, r) slices
                        for g in range(NG):
                            pi = prod.tile([128, 4, D, J], F16, tag="pi")
                            nc.vector.tensor_tensor(
                                pi[:],
                                C[:, g],
                                c_sb[:, g, :, None, :].broadcast_to(
                                    [128, 4, D, J]
                                ),
                                op=Alu.mult,
                            )
                            for r in range(4):
                                nc.tensor.matmul(
                                    s_ps[:],
                                    sel,
                                    pi[:, r].rearrange("p d j -> p (d j)"),
                                    start=(g == 0 and r == 0),
                                    stop=(g == NG - 1 and r == 3),
                                    skip_group_check=True,
                                )
                        nc.scalar.copy(s_loc[:], s_ps[:])
                        s_glob = all_reduce_s(it)

                    v_f = squash(s_glob)

                    if it < ROUTINGS - 1:
                        # replicate v over partitions via PE, then agreement:
                        # bl += sum_d C * v_rep, accumulated on the PE via the
                        # identity stationary (16 d-slices per g-block).
                        vr_ps = psB.tile([128, JD], F32, tag="vr_ps")
                        nc.tensor.matmul(
                            vr_ps[:], selT, v_f.rearrange("b d j -> b (d j)")
                        )
                        nc.scalar.copy(
                            v_rep.rearrange("p d j -> p (d j)"), vr_ps[:]
                        )
                        for blk in range(4):
                            pi2 = prod.tile([128, 4, 4, D, J], F16, tag="pi2")
                            nc.vector.tensor_tensor(
                                pi2[:],
                                C[:, 4 * blk : 4 * blk + 4],
                                v_rep[:, None, None, :, :].broadcast_to(
                                    [128, 4, 4, D, J]
                                ),
                                op=Alu.mult,
                            )
                            for d in range(D):
                                nc.tensor.matmul(
                                    bl_ps[:, 4 * blk : 4 * blk + 4, :, :],
                                    ident,
                                    pi2[:, :, :, d, :],
                                    start=(it == 0 and d == 0),
                                    stop=(d == D - 1),
                                    skip_group_check=True,
                                )
                    else:
                        # final output: reorder (d, j) -> (j, d) and store
                        v_jd = sp.tile([B, J, D], F32, tag="v_jd")
                        nc.vector.tensor_copy(
                            v_jd[:], v_f.rearrange("b d j -> b j d")
                        )
                        if DEBUG_STAGE != "phase1":
                            nc.sync.dma_start(v_out[:], v_jd[:])

    nc.compile()
    return nc


def _prep_inputs(x, W):
    """Per-core host-side sharding + layout prep (fp16)."""
    x16 = x.astype(np.float16)
    W16 = W.astype(np.float16)
    in_maps = []
    for m in range(NCORES):
        lo, hi = m * I_LOC, (m + 1) * I_LOC
        Wc = W16[:, lo:hi]                     # [J, 256, D, K]
        # wm[(r,c,k), g, (d,j)] = Wc[j, g*16+r*4+c, d, k]
        Wm = Wc.reshape(J, NG, 4, 4, D, K)     # j, g, r, c, d, k
        wm = np.ascontiguousarray(Wm.transpose(2, 3, 5, 1, 4, 0)).reshape(
            128, NG, JD
        )
        # ws0[(i16,k), ch, (d,j)] = Wc[j, ch*16+i16, d, k]
        Ws = Wc.reshape(J, NCH, 16, D, K)      # j, ch, i16, d, k
        ws0 = np.ascontiguousarray(Ws.transpose(2, 4, 1, 3, 0)).reshape(
            128, NCH, JD
        )
        xc = x16[:, lo:hi, :]                  # [B, 256, K]
        # xsel[(r,c,k), g, (c',b)] = x[b, g*16+r*4+c, k] * [c == c']
        xg = xc.reshape(B, NG, 4, 4, K)        # b, g, r, c, k
        xsel = np.zeros((4, 4, K, NG, 4, B), np.float16)
        for c in range(4):
            xsel[:, c, :, :, c, :] = xg.transpose(2, 3, 4, 1, 0)[:, c]
        xsel = xsel.reshape(128, NG, 128)
        # xs0[(i16,k), ch, b] = x[b, ch*16+i16, k] / J
        xs = xc.reshape(B, NCH, 16, K).transpose(2, 3, 1, 0)  # i16, k, ch, b
        xs0 = np.ascontiguousarray(xs).reshape(128, NCH, B) * np.float16(1.0 / J)
        in_maps.append({"wm": wm, "ws0": ws0, "xsel": xsel, "xs0": xs0})
    return in_maps


def run(inputs, trace=False):
    if "nc" not in _CACHE:
        _CACHE["nc"] = _build()
    nc = _CACHE["nc"]
    in_maps = _prep_inputs(inputs["x"], inputs["W"])
    bkr = run_bass_kernel_spmd(
        nc, in_maps, core_ids=list(range(NCORES)), trace=trace
    )
    out = bkr.results[0]["v"].astype(np.float32)
    return out, bkr


def kernel(x, W):
    out, _ = run({"x": np.asarray(x), "W": np.asarray(W)})
    return out
